# revision 22
# baseline (speedup 1.0000x reference)
"""CODAPromptPool kernel for 8 Trainium2 NeuronCores.

Reference computation (per batch element b):
    query  = mean(x[b], axis=0)                      # [D]
    sim    = l2norm(query) @ l2norm(e_keys).T        # [POOL]
    top4   = top_k(sim, 4) indices (descending)
    out[b] = concat([g_prompts[task_id],             # rows 0..7
                     e_prompts[top4].reshape(32, D), # rows 8..39
                     cls_token,                      # row 40
                     x[b]], axis=0)                  # rows 41..2088

Sharding: data-parallel over batch (64 / 8 cores = 8 per core); the pool /
keys / g / cls are replicated. The kernel is HBM-bound by the x copy
(48 MiB in + 49 MiB out per core); x streams through SBUF once, each tile
feeding both the per-batch seq-sum (for routing) and the output copy.

Layout: each half-batch (1024 rows = 3 MiB, contiguous in both x and
out) moves as ONE dma shaped [128, 6144] - 128 descriptors of 24.6 KiB
(contiguous per partition) instead of 1024 x 3 KiB (at 3 KiB/packet the
SDMA engines ran at ~88% of line rate; big descriptors reach ~26.6
GB/s/engine = full line rate). In-stream rides the sync HWDGE ring,
out-stream the scalar ring; the first in-DMAs are emitted before the
routing preamble so the bulk stream ramps immediately, and the last 3
chunk-writes are deferred to the end to cover the routing chain.
Routing notes:
  * top-k ranking is invariant to positive per-row scaling, so neither
    the division by S (mean) nor the query l2-normalization is needed -
    only the keys must be normalized.
  * the gather of selected prompt blocks uses indirect DMA with uint32
    indices from the DVE max8 instruction, expanded 4x (idx*4+j against
    the [POOL*4, 1536]-reshaped pool) so the gather and the single
    e-block write cover all 128 partitions - SBUF-side descriptors are
    partition-pinned to their port's SDMA engine, so a [32, 6144] layout
    would put all 786 KB on half the engines and straggle the tail.
"""

import numpy as np

import concourse.bacc as bacc
import concourse.bass as bass
import concourse.mybir as mybir
from concourse import bass_utils
from concourse._compat import get_trn_type
from concourse.masks import make_identity
from concourse.tile import TileContext

F32 = mybir.dt.float32
U32 = mybir.dt.uint32

NCORES = 8
B, S, D = 64, 2048, 768
BC = B // NCORES                 # batches per core
POOL, L, TOPK = 32, 8, 4
E_OFF = L                        # selected blocks start row
CLS_ROW = L + TOPK * L           # 40
X_OFF = CLS_ROW + 1              # 41
OUTS = X_OFF + S                 # 2089
EPS = 1e-12
P = 128
ESPLIT = 4                       # e_prompts row split for engine balance
ESUB = L * D // ESPLIT           # 1536 elems per sub-row

PROFILE = False                  # test harness sets True for NTFF tracing
LAST_RESULT = None               # BassKernelResults of the last run


def build(bc=BC, s=S, debug=False, ch=1024, defer_h=3, xp_bufs=3):
    """ch: rows of x per bulk DMA (must divide s, multiple of P).
    defer_h: trailing chunk-writes kept in SBUF and drained during the
    routing chain at the end. xp_bufs: streaming buffer depth."""
    assert s % ch == 0 and ch % P == 0
    nt = s // ch                 # chunks per batch
    cw = ch // P                 # x rows per partition per chunk
    fw = cw * D                  # chunk free size (elems per partition)
    ndc = D // P                 # 6 D-chunks of 128
    outs = X_OFF + s
    x = mybir.AxisListType.X
    n_half = bc * nt
    def_start = n_half - defer_h

    nc = bacc.Bacc(get_trn_type() or "TRN2", target_bir_lowering=False, debug=debug)
    x_h = nc.declare_dram_parameter("x", [bc, s, D], F32, isOutput=False)
    ep_h = nc.declare_dram_parameter(
        "e_prompts", [POOL * ESPLIT, ESUB], F32, isOutput=False
    )
    ek_h = nc.declare_dram_parameter("e_keys", [POOL, D], F32, isOutput=False)
    g_h = nc.declare_dram_parameter("g_rep", [bc, L, D], F32, isOutput=False)
    cls_h = nc.declare_dram_parameter("cls_rep", [bc, 1, D], F32, isOutput=False)
    out_h = nc.declare_dram_parameter("out", [bc, outs, D], F32, isOutput=True)

    def x_src(b, h):
        return x_h[b, h * ch : (h + 1) * ch, :].rearrange("(p r) d -> p (r d)", p=P)

    def out_dst(b, h):
        return out_h[b, X_OFF + h * ch : X_OFF + (h + 1) * ch, :].rearrange(
            "(p r) d -> p (r d)", p=P
        )

    with TileContext(nc) as tc:
        with (
            tc.tile_pool(name="consts", bufs=1) as consts,
            tc.tile_pool(name="xp", bufs=xp_bufs) as xp,
            tc.tile_pool(name="xdef", bufs=1) as xdef,
            tc.tile_pool(name="accp", bufs=2) as accp,
            tc.tile_pool(name="rt", bufs=2) as rt,
            tc.tile_pool(name="gp", bufs=1) as gp,
            tc.tile_pool(name="ps", bufs=2, space="PSUM") as ps,
            tc.tile_pool(name="ps1", bufs=1, space="PSUM") as ps1,
        ):
            # Kick off the bulk x stream first so the SDMA engines ramp
            # immediately; the routing preamble below overlaps with it.
            tiles = {}

            def alloc_tile(g):
                if g >= def_start:
                    return xdef.tile([P, fw], F32, tag=f"bdef_{g}", name=f"bdef_{g}")
                return xp.tile([P, fw], F32, tag="xt", name="xt")

            for h in range(nt):
                t = alloc_tile(h)
                tiles[(0, h)] = t
                eng = nc.sync if h % 2 == 0 else nc.scalar
                eng.dma_start(t[:], x_src(0, h))

            # Routing-independent header rows, straight DRAM->DRAM (SWDGE,
            # off the HWDGE rings that carry the bulk stream).
            nc.gpsimd.dma_start(out_h[:, 0:L, :], g_h[:])
            nc.gpsimd.dma_start(out_h[:, CLS_ROW : CLS_ROW + 1, :], cls_h[:])

            ident = consts.tile([P, P], F32)
            make_identity(nc, ident[:])

            # Normalized keys, transposed to [D-chunk partitions, POOL].
            keys = consts.tile([POOL, D], F32)
            nc.gpsimd.dma_start(keys[:], ek_h[:])
            sq = consts.tile([POOL, D], F32)
            nc.vector.tensor_mul(sq[:], keys[:], keys[:])
            n2 = consts.tile([POOL, 1], F32)
            nc.vector.reduce_sum(n2[:], sq[:], axis=x)
            eps = consts.tile([POOL, 1], F32)
            nc.vector.memset(eps[:], EPS)
            nrm = consts.tile([POOL, 1], F32)
            nc.scalar.activation(
                nrm[:], n2[:], mybir.ActivationFunctionType.Sqrt, bias=eps[:, 0:1]
            )
            rk = consts.tile([POOL, 1], F32)
            nc.vector.reciprocal(rk[:], nrm[:])
            kn = consts.tile([P, D], F32)
            nc.vector.memset(kn[:], 0.0)
            nc.vector.tensor_scalar_mul(kn[0:POOL, :], keys[:], rk[:, 0:1])
            knT = consts.tile([P, ndc * POOL], F32)
            for c in range(ndc):
                pt = ps.tile([P, P], F32, tag="tp")
                nc.tensor.transpose(pt[:], kn[:, bass.ts(c, P)], ident[:])
                nc.vector.tensor_copy(knT[:, bass.ts(c, POOL)], pt[:, 0:POOL])

            # Stream x: per chunk one big DMA in, one big DMA out, and DVE
            # adds folding the chunk into the per-batch accumulator. The
            # last defer_h chunk-writes are emitted after the routing ops
            # so they drain while the routing chain runs.
            qt_all = consts.tile([P, ndc * bc], F32)
            for b in range(bc):
                acc = accp.tile([P, D], F32, tag="acc")
                first = True
                for h in range(nt):
                    g = b * nt + h
                    if (b, h) in tiles:
                        t = tiles[(b, h)]
                    else:
                        t = alloc_tile(g)
                        tiles[(b, h)] = t
                        nc.sync.dma_start(t[:], x_src(b, h))
                    if g < def_start:
                        nc.scalar.dma_start(out_dst(b, h), t[:])
                    for k in range(cw):
                        sl = t[:, k * D : (k + 1) * D]
                        if first:
                            nxt = t[:, D : 2 * D]
                            nc.vector.tensor_add(acc[:], sl, nxt)
                            first = False
                        elif h == 0 and k == 1:
                            continue
                        else:
                            nc.vector.tensor_add(acc[:], acc[:], sl)
                # Partition-reduce acc via PE transpose + free-axis sum.
                for c in range(ndc):
                    pt = ps.tile([P, P], F32, tag="tp")
                    nc.tensor.transpose(pt[:], acc[:, bass.ts(c, P)], ident[:])
                    nc.vector.reduce_sum(qt_all[:, c * bc + b : c * bc + b + 1], pt[:], axis=x)

            # Batched routing for all bc batches at once.
            sps = ps1.tile([bc, POOL], F32, tag="s")
            for c in range(ndc):
                nc.tensor.matmul(
                    sps[:],
                    lhsT=qt_all[:, bass.ts(c, bc)],
                    rhs=knT[:, bass.ts(c, POOL)],
                    start=(c == 0),
                    stop=(c == ndc - 1),
                )
            s_sb = rt.tile([bc, POOL], F32, tag="ssb")
            nc.vector.tensor_copy(s_sb[:], sps[:])
            mx = rt.tile([bc, 8], F32, tag="mx")
            ix = rt.tile([bc, 8], U32, tag="ix")
            nc.vector.max_with_indices(mx[:], ix[:], s_sb[:])
            # Expand the 32 selected pool rows to 128 sub-row indices
            # (idx*4+j, k-major) against the [POOL*4, 1536]-reshaped pool,
            # so the gather and the e-block write cover all 128 partitions
            # = all 16 partition-pinned SDMA engines instead of 8.
            ixf = rt.tile([bc, TOPK], F32, tag="ixf")
            nc.vector.tensor_copy(ixf[:], ix[:, 0:TOPK])
            ix4 = rt.tile([bc, TOPK], F32, tag="ix4")
            nc.vector.tensor_scalar_mul(ix4[:], ixf[:], float(ESPLIT))
            jc = rt.tile([bc, ESPLIT], F32, tag="jc")
            for j in range(ESPLIT):
                nc.vector.memset(jc[:, j : j + 1], float(j))
            ix16f = rt.tile([bc, ESPLIT * TOPK], F32, tag="ix16f")
            for k in range(TOPK):
                nc.vector.tensor_scalar_add(
                    ix16f[:, k * ESPLIT : (k + 1) * ESPLIT], jc[:], ix4[:, k : k + 1]
                )
            ix16 = rt.tile([bc, ESPLIT * TOPK], U32, tag="ix16")
            nc.vector.tensor_copy(ix16[:], ix16f[:])
            idx128 = rt.tile([bc * ESPLIT * TOPK, 1], U32, tag="idx128")
            nc.gpsimd.dma_start(idx128[:], ix16[:])
            gth = gp.tile([bc * ESPLIT * TOPK, ESUB], F32, tag="gth")
            nc.gpsimd.indirect_dma_start(
                out=gth[:],
                out_offset=None,
                in_=ep_h[:],
                in_offset=bass.IndirectOffsetOnAxis(ap=idx128[:, 0:1], axis=0),
            )

            # Deferred chunk writes, split across both HWDGE rings so they
            # drain at full rate while the routing chain (max8 -> index
            # expand/spread -> indirect gather) runs. The gather write goes
            # last so it can't head-of-line-block them.
            for i, g in enumerate(range(def_start, n_half)):
                b, h = divmod(g, nt)
                eng = nc.scalar if i % 2 == 0 else nc.sync
                eng.dma_start(out_dst(b, h), tiles[(b, h)][:])
            e_dst = out_h[:, E_OFF : E_OFF + TOPK * L, :].rearrange(
                "b (k j l) d -> b (k j) (l d)", k=TOPK, j=ESPLIT
            )
            nc.sync.dma_start(e_dst, gth[:])

    nc.compile()
    return nc


_NC_CACHE: dict = {}


def _get_nc(bc=BC, s=S):
    key = (bc, s)
    if key not in _NC_CACHE:
        _NC_CACHE[key] = build(bc, s)
    return _NC_CACHE[key]


def kernel(x, g_prompts, e_prompts, e_keys, cls_token, task_id):
    global LAST_RESULT
    nc = _get_nc()
    tid = int(np.asarray(task_id))
    x = np.ascontiguousarray(np.asarray(x, dtype=np.float32))
    g_rep = np.ascontiguousarray(
        np.broadcast_to(np.asarray(g_prompts, np.float32)[tid][None], (BC, L, D))
    )
    cls_rep = np.ascontiguousarray(
        np.broadcast_to(np.asarray(cls_token, np.float32).reshape(1, 1, D), (BC, 1, D))
    )
    ep = np.ascontiguousarray(
        np.asarray(e_prompts, np.float32).reshape(POOL * ESPLIT, ESUB)
    )
    ek = np.ascontiguousarray(np.asarray(e_keys, np.float32))

    in_maps = [
        {
            "x": x[c * BC : (c + 1) * BC],
            "e_prompts": ep,
            "e_keys": ek,
            "g_rep": g_rep,
            "cls_rep": cls_rep,
        }
        for c in range(NCORES)
    ]
    res = bass_utils.run_bass_kernel_spmd(
        nc, in_maps, list(range(NCORES)), trace=PROFILE
    )
    LAST_RESULT = res
    return np.concatenate([res.results[c]["out"] for c in range(NCORES)], axis=0)
